# revision 1
# baseline (speedup 1.0000x reference)
"""Trainium2 Bass kernel v3 for nn_NoGraphLayer (single-query neighbor attention + FFN).

Full (unsharded) inputs in, full output out. Pure data-parallel over 8 cores
(4096 anchors each), 32 blocks of 128 anchors per core.

Strategy:
  - x_nei pre-transposed + pre-cast to fp8e4m3 on the HOST into
    [block, d-chunk, d, (k b)] layout: no on-chip transposes of the big
    tensor, 4x less DMA than fp32. x_anc pre-cast to bf16.
  - K/V projections are fp8 DoubleRow matmuls (256-deep contraction per
    instruction at 0.5 cyc/row). attn@V k-summation uses stacked-identity
    fp8 DoubleRow matmuls (2 k's per instruction).
  - Residual adds run on PE as identity-matmul accumulation into the same
    PSUM group (keeps DVE free); ACT copies the sums out.
  - Scores: per-4k-group routing ('d' DVE-direct from psum, 'a' ACT-escape
    to bf16 + DVE 2x mul, 'p' ACT-escape + Pool mul); dk-reduce always DVE
    (only free-axis reduce engine); Pool-routed groups' reduces are emitted
    last to avoid DVE head-of-line blocking.
  - V phase: DVE-direct pv groups FIRST (ready earliest), Pool groups
    (via ACT bf16 escape) after, so DVE can run ahead into the next block.
  - LN via bn_stats/bn_aggr; rsqrt = exp(-0.5*ln(var+eps)) on ACT sharing
    the patched natural_log_exp_and_others table with softmax exp.
  - FFN batched per 8-block superblock: all LN2+transposes first, then all
    FF1/gelu/FF2 (2 ACT table swaps per superblock). FF1 computed
    output-transposed, bias via ones-row matmuls, so no fgT transposes.
  - PSUM budget (8 banks): kv pool 4, vv pool (shared with FFN gp/f2p) 2,
    up 1, smalls (transposes/qp/ao) 1.
"""

import math
from contextlib import ExitStack

import numpy as np
import ml_dtypes

import concourse.bass as bass
import concourse.tile as tile
from concourse.tile_rust import add_dep_helper
from concourse import bacc
from concourse import mybir

F32 = mybir.dt.float32
BF16 = mybir.dt.bfloat16
FP8 = mybir.dt.float8e4
AX = mybir.AxisListType
ALU = mybir.AluOpType
ACT = mybir.ActivationFunctionType
DR = mybir.MatmulPerfMode.DoubleRow

B, K, D, H, FF = 32768, 32, 256, 8, 1024
DK = D // H
P = 128
NCORES = 8
BC = B // NCORES            # anchors per core
NBLK = BC // P              # 32 blocks per core
G = 16                      # blocks per superblock (FFN batching)
KG = 4                      # k's per K-phase psum group
NKG = K // KG               # 8 K-groups
VG = 2                      # k's per V-phase psum group
NVG = K // VG               # 16 V-groups
EPS = 1e-5

# ---- engine split knobs (tuned via TimelineSim) ----
KROUTES = "appappap"  # per K-group: d=DVE-direct, a=ACT+DVE-2x, p=ACT+Pool
PV_DVE_N = 16   # first N V-groups multiplied on DVE (psum-direct)
LN_ON_POOL = True           # LN (x-mu)*rs applies on gpsimd
ATTN_ON_POOL = False        # attn = e*zr mul engine
FFJ = 2                     # block index in next superblock where FF pass is emitted
KSPLIT = 2                  # K-groups emitted before the previous block's B stage

# bf16 weight pack layout (columns)
WQ_OFF = 0                  # [2, 256]
WO_OFF = 512                # [2, 256]
W1_OFF = 1024               # [2, 8, 128]
W2_OFF = 3072               # [8, 256]
IDN_OFF = 5120              # [128]
WBF_COLS = 5248
# fp8 weight pack layout
WK8_OFF = 0                 # [2, 256]
WV8_OFF = 512               # [2, 256]
IDN2_OFF = 1024             # [2, 128]
W8_COLS = 1280
# bf16 row-vector pack
ONES_OFF = 0                # [128]
BQ_OFF = 128                # [256]
B2_OFF = 384                # [256]
B1_OFF = 640                # [8, 128]
VROW_COLS = 1664


def emit_layer(tc, io, n_blocks):
    nc = tc.nc
    with ExitStack() as ctx:
        const = ctx.enter_context(tc.tile_pool(name="const", bufs=1))
        xpool = ctx.enter_context(tc.tile_pool(name="xin", bufs=4))
        sb = ctx.enter_context(tc.tile_pool(name="sb", bufs=4))
        sb3 = ctx.enter_context(tc.tile_pool(name="sb3", bufs=6))
        xsp = ctx.enter_context(tc.tile_pool(name="xsp", bufs=G + 6))
        ps_kv = ctx.enter_context(tc.tile_pool(name="pskv", bufs=2, space="PSUM"))
        ps_vv = ctx.enter_context(tc.tile_pool(name="psvv", bufs=3, space="PSUM"))
        ps_u = ctx.enter_context(tc.tile_pool(name="psu", bufs=1, space="PSUM"))

        w8 = const.tile([P, W8_COLS], FP8, tag="w8")
        nc.sync.dma_start(w8[:], io["w8"])
        vrow = const.tile([1, VROW_COLS], BF16, tag="vrow")
        nc.sync.dma_start(vrow[:], io["vrow"])
        wbf = const.tile([P, WBF_COLS], BF16, tag="wbf")
        wbf_pending = [True]

        def load_wbf():
            # emitted lazily after block 0's input DMAs so they lead SP's queue
            if wbf_pending[0]:
                nc.sync.dma_start(wbf[:], io["wbf"])
                wbf_pending[0] = False

        wq = wbf[:, WQ_OFF:WQ_OFF + 512].rearrange("p (c n) -> p c n", c=2)
        wo = wbf[:, WO_OFF:WO_OFF + 512].rearrange("p (c n) -> p c n", c=2)
        w1 = wbf[:, W1_OFF:W1_OFF + 2048].rearrange("p (c f n) -> p c f n", c=2, f=8)
        w2 = wbf[:, W2_OFF:W2_OFF + 2048].rearrange("p (f n) -> p f n", f=8)
        idn = wbf[:, IDN_OFF:IDN_OFF + P]
        wk8 = w8[:, WK8_OFF:WK8_OFF + 512].rearrange("p (c n) -> p c n", c=2)
        wv8 = w8[:, WV8_OFF:WV8_OFF + 512].rearrange("p (c n) -> p c n", c=2)
        idn2 = w8[:, IDN2_OFF:IDN2_OFF + 256].rearrange("p (c n) -> p c n", c=2)
        ones1 = vrow[:, ONES_OFF:ONES_OFF + P]
        bq = vrow[:, BQ_OFF:BQ_OFF + 256]
        b2 = vrow[:, B2_OFF:B2_OFF + 256]
        b1 = vrow[:, B1_OFF:B1_OFF + 1024].rearrange("p (f n) -> p f n", f=8)

        def rsqrt_stats(src_t, work_tag):
            # bn_stats/aggr -> (bna [p,2]=(mean,var), rs [p,1]=rsqrt(var+eps))
            bns = sb.tile([P, 6], F32, tag=work_tag + "_bns")
            nc.vector.bn_stats(bns[:], src_t[:])
            bna = sb.tile([P, 2], F32, tag=work_tag + "_bna")
            nc.vector.bn_aggr(bna[:], bns[:])
            ve = sb.tile([P, 1], F32, tag=work_tag + "_ve")
            nc.vector.tensor_scalar(out=ve[:], in0=bna[:, 1:2], scalar1=EPS,
                                    scalar2=None, op0=ALU.add)
            lnv = sb.tile([P, 1], F32, tag=work_tag + "_lnv")
            nc.scalar.activation(lnv[:], ve[:], ACT.Ln)
            rs = sb.tile([P, 1], F32, tag=work_tag + "_rs")
            nc.scalar.activation(rs[:], lnv[:], ACT.Exp, scale=-0.5)
            return bna, rs

        def ln_apply(dst, src, bna, rs):
            eng = nc.gpsimd if LN_ON_POOL else nc.vector
            eng.tensor_scalar(out=dst[:], in0=src[:], scalar1=bna[:, 0:1],
                              scalar2=rs[:], op0=ALU.subtract, op1=ALU.mult)

        def transpose2(src_bf, dst_tag, pool=None):
            # [128, 256] bf16 -> transposed [128, 256] bf16 via PE + ACT copy
            tps = ps_vv.tile([P, 128], F32, tag="vv")
            tp = tps[:].bitcast(BF16)
            for j in range(2):
                nc.tensor.transpose(tp[:, j * P:(j + 1) * P],
                                    src_bf[:, j * P:(j + 1) * P], idn)
            dst = (pool or sb).tile([P, 256], BF16, tag=dst_tag)
            nc.scalar.copy(dst[:], tp[:, 0:256])
            return dst

        xs_tiles = {}

        def emit_stage_a(i):
            last_kvs = None
            # ---- loads ----
            xab = xpool.tile([P, D], BF16, tag="xab")
            nc.sync.dma_start(xab[:], io["xab"][i * P:(i + 1) * P, :])
            xt8 = xpool.tile([P, 2, K * P], FP8, tag="xt8")
            nc.sync.dma_start(
                xt8[:],
                io["xt8"][i * 2 * P:(i + 1) * 2 * P, :]
                .rearrange("(c p) n -> p c n", c=2))
            load_wbf()

            # ---- LN1 + Q ----
            bna, rs = rsqrt_stats(xab, "ln1")
            lnx = sb.tile([P, D], BF16, tag="lnx")
            ln_apply(lnx, xab, bna, rs)
            lnxT = transpose2(lnx, "lnxT")
            qp = ps_kv.tile([P, 256], F32, tag="kv")
            for kt in range(2):
                nc.tensor.matmul(qp[:], lnxT[:, kt * P:(kt + 1) * P],
                                 wq[:, kt, :], start=(kt == 0), stop=False)
            nc.tensor.matmul(qp[:], ones1, bq, start=False, stop=True)
            qs = sb.tile([P, D], BF16, tag="qs")
            nc.scalar.copy(qs[:], qp[:])

            # ---- K phase: scores ----
            scoresN = sb.tile([P, K * H], BF16, tag="scores")
            qsb = qs[:].rearrange("p (o n) -> p o n", o=1)
            late_reds = []
            for g in range(KSPLIT):
                route = KROUTES[g]
                kv = ps_kv.tile([P, KG * D], F32, tag="kv")
                for kk in range(KG):
                    k = g * KG + kk
                    nc.tensor.matmul(
                        kv[:, kk * D:(kk + 1) * D],
                        xt8[:, :, k * P:(k + 1) * P],
                        wk8[:], start=True, stop=True, perf_mode=DR)
                pr = sb3.tile([P, KG * D], BF16, tag="pr")
                prv = pr[:].rearrange("p (k n) -> p k n", n=D)
                red_out = scoresN[:, g * KG * H:(g + 1) * KG * H]
                red_in = pr[:].rearrange("p (kh dk) -> p kh dk", dk=DK)
                if route == "d":
                    nc.vector.tensor_mul(
                        prv, kv[:].rearrange("p (k n) -> p k n", n=D),
                        qsb.to_broadcast((P, KG, D)))
                    with nc.allow_low_precision("bf16 scores"):
                        nc.vector.tensor_reduce(red_out, red_in,
                                                axis=AX.X, op=ALU.add)
                else:
                    kvs = sb3.tile([P, KG * D], BF16, tag="kvs")
                    last_kvs = nc.scalar.copy(kvs[:], kv[:])
                    eng = nc.vector if route == "a" else nc.gpsimd
                    eng.tensor_mul(
                        prv, kvs[:].rearrange("p (k n) -> p k n", n=D),
                        qsb.to_broadcast((P, KG, D)))
                    if route == "a":
                        with nc.allow_low_precision("bf16 scores"):
                            nc.vector.tensor_reduce(red_out, red_in,
                                                    axis=AX.X, op=ALU.add)
                    else:
                        late_reds.append((red_out, red_in))
            return dict(i=i, xab=xab, xt8=xt8, scoresN=scoresN,
                        last_kvs=last_kvs, qsb=qsb, late_reds=late_reds)

        def emit_stage_a2(stg):
            # K-groups [KSPLIT, NKG) emitted after the previous block's
            # B stage, so V(i-1) PE fills aren't queued behind all of
            # K(i)'s fills in PE's in-order stream.
            i, xt8, scoresN = stg["i"], stg["xt8"], stg["scoresN"]
            qsb, late_reds = stg["qsb"], stg["late_reds"]
            last_kvs = stg["last_kvs"]
            for g in range(KSPLIT, NKG):
                route = KROUTES[g]
                kv = ps_kv.tile([P, KG * D], F32, tag="kv")
                for kk in range(KG):
                    k = g * KG + kk
                    nc.tensor.matmul(
                        kv[:, kk * D:(kk + 1) * D],
                        xt8[:, :, k * P:(k + 1) * P],
                        wk8[:], start=True, stop=True, perf_mode=DR)
                pr = sb3.tile([P, KG * D], BF16, tag="pr")
                prv = pr[:].rearrange("p (k n) -> p k n", n=D)
                red_out = scoresN[:, g * KG * H:(g + 1) * KG * H]
                red_in = pr[:].rearrange("p (kh dk) -> p kh dk", dk=DK)
                if route == "d":
                    nc.vector.tensor_mul(
                        prv, kv[:].rearrange("p (k n) -> p k n", n=D),
                        qsb.to_broadcast((P, KG, D)))
                    with nc.allow_low_precision("bf16 scores"):
                        nc.vector.tensor_reduce(red_out, red_in,
                                                axis=AX.X, op=ALU.add)
                else:
                    kvs = sb3.tile([P, KG * D], BF16, tag="kvs")
                    last_kvs = nc.scalar.copy(kvs[:], kv[:])
                    eng = nc.vector if route == "a" else nc.gpsimd
                    eng.tensor_mul(
                        prv, kvs[:].rearrange("p (k n) -> p k n", n=D),
                        qsb.to_broadcast((P, KG, D)))
                    if route == "a":
                        with nc.allow_low_precision("bf16 scores"):
                            nc.vector.tensor_reduce(red_out, red_in,
                                                    axis=AX.X, op=ALU.add)
                    else:
                        late_reds.append((red_out, red_in))
            for red_out, red_in in late_reds:
                with nc.allow_low_precision("bf16 scores"):
                    nc.vector.tensor_reduce(red_out, red_in,
                                            axis=AX.X, op=ALU.add)
            stg["last_kvs"] = last_kvs

        def emit_stage_b(stg, next_kvs=None):
            i, xab, xt8, scoresN = stg["i"], stg["xab"], stg["xt8"], stg["scoresN"]
            # ---- softmax over k ----
            e = sb.tile([P, K * H], BF16, tag="e")
            nc.scalar.activation(e[:], scoresN[:], ACT.Exp)
            z = sb.tile([P, H], F32, tag="z")
            with nc.allow_low_precision("z sum"):
                nc.vector.tensor_reduce(
                    z[:], e[:].rearrange("p (k h) -> p h k", h=H),
                    axis=AX.X, op=ALU.add)
            zr = sb.tile([P, H], F32, tag="zr")
            nc.vector.reciprocal(zr[:], z[:])
            attn = sb.tile([P, K * H], BF16, tag="attn")
            attn_eng = nc.gpsimd if ATTN_ON_POOL else nc.vector
            attn_eng.tensor_mul(
                attn[:].rearrange("p (k h) -> p k h", h=H),
                e[:].rearrange("p (k h) -> p k h", h=H),
                zr[:].rearrange("p (o h) -> p o h", o=1)
                    .to_broadcast((P, K, H)))

            # ---- V phase: attn-weighted sum (DVE groups first) ----
            up = ps_u.tile([P, D], F32, tag="up")
            for g in range(NVG):
                vv = ps_vv.tile([P, VG * D], F32, tag="vv")
                for kk in range(VG):
                    k = g * VG + kk
                    nc.tensor.matmul(
                        vv[:, kk * D:(kk + 1) * D],
                        xt8[:, :, k * P:(k + 1) * P],
                        wv8[:], start=True, stop=True, perf_mode=DR)
                pv = sb3.tile([P, VG * D], FP8, tag="pv")
                attn_b = (attn[:].rearrange("p (k h) -> p k h", h=H)
                          [:, g * VG:(g + 1) * VG, :]
                          .to_broadcast((P, VG, H, DK)))
                pview = pv[:].rearrange("p (k h dk) -> p k h dk", h=H, dk=DK)
                if g < PV_DVE_N:
                    nc.vector.tensor_mul(
                        pview,
                        vv[:].rearrange("p (k h dk) -> p k h dk", h=H, dk=DK),
                        attn_b)
                else:
                    vvs = sb3.tile([P, VG * D], BF16, tag="vvs")
                    nc.scalar.copy(vvs[:], vv[:])
                    nc.gpsimd.tensor_mul(
                        pview,
                        vvs[:].rearrange("p (k h dk) -> p k h dk", h=H, dk=DK),
                        attn_b)
                nc.tensor.matmul(
                    up[:], idn2[:],
                    pv[:].rearrange("p (c n) -> p c n", c=2),
                    start=(g == 0), stop=(g == NVG - 1),
                    perf_mode=DR)

            # ---- W_o + residual (residual add on PE) ----
            us = sb.tile([P, D], BF16, tag="us")
            us_inst = nc.scalar.copy(us[:], up[:])
            if next_kvs is not None:
                # Pin ACT order: the next block's Kv escapes must precede this
                # block's tail copies, else ACT head-of-line blocks the next
                # block's DVE score work behind the Pool pv tail.
                add_dep_helper(_inst(us_inst), _inst(next_kvs),
                               reason="ACT: next-block kv escapes before tail")
            uT = transpose2(us, "uT")
            ao = ps_vv.tile([P, 256], F32, tag="vv")
            for kt in range(2):
                nc.tensor.matmul(ao[:], uT[:, kt * P:(kt + 1) * P],
                                 wo[:, kt, :], start=(kt == 0), stop=False)
            nc.tensor.matmul(ao[:], idn, xab[:], start=False, stop=True)
            xsb = xsp.tile([P, D], BF16, tag="xs")
            nc.scalar.copy(xsb[:], ao[:])
            xs_tiles[i] = xsb

        def emit_ffn_ln2(ffn_blocks, dep_inst):
            # All LN2s + transposes first (Ln/Exp table).
            tc.cur_priority += 1000000
            hT_tiles = {}
            for i in ffn_blocks:
                xsb = xs_tiles[i]
                bns = sb.tile([P, 6], F32, tag="ln2_bns")
                bn_inst = nc.vector.bn_stats(bns[:], xsb[:])
                if dep_inst is not None:
                    add_dep_helper(_inst(bn_inst), _inst(dep_inst),
                                   reason="defer FFN behind next superblock")
                bna2 = sb.tile([P, 2], F32, tag="ln2_bna")
                nc.vector.bn_aggr(bna2[:], bns[:])
                ve = sb.tile([P, 1], F32, tag="ln2_ve")
                nc.vector.tensor_scalar(out=ve[:], in0=bna2[:, 1:2], scalar1=EPS,
                                        scalar2=None, op0=ALU.add)
                lnv = sb.tile([P, 1], F32, tag="ln2_lnv")
                nc.scalar.activation(lnv[:], ve[:], ACT.Ln)
                rs2 = sb.tile([P, 1], F32, tag="ln2_rs")
                nc.scalar.activation(rs2[:], lnv[:], ACT.Exp, scale=-0.5)
                hs = sb.tile([P, D], BF16, tag="hs")
                ln_apply(hs, xsb, bna2, rs2)
                hT_tiles[i] = transpose2(hs, "hT", pool=xsp)
            tc.cur_priority -= 1000000
            return hT_tiles

        def emit_ffn_ff(ffn_blocks, hT_tiles):
            # FF1+gelu for all blocks first (PE never waits on gelu), then
            # all FF2s (their gt inputs are ready by then).
            tc.cur_priority += 1000000
            gt_tiles = {}
            for i in ffn_blocks:
                hT = hT_tiles[i]
                gt = xsp.tile([P, FF], BF16, tag="gt")
                for half in range(2):
                    gp = ps_vv.tile([P, 512], F32, tag="vv")
                    for q in range(4):
                        f = half * 4 + q
                        for c in range(2):
                            nc.tensor.matmul(
                                gp[:, q * P:(q + 1) * P],
                                w1[:, c, f, :], hT[:, c * P:(c + 1) * P],
                                start=(c == 0), stop=False)
                        nc.tensor.matmul(
                            gp[:, q * P:(q + 1) * P],
                            b1[:, f, :], ones1,
                            start=False, stop=True)
                    nc.scalar.activation(gt[:, half * 512:(half + 1) * 512],
                                         gp[:], ACT.Gelu)
                gt_tiles[i] = gt
            for i in ffn_blocks:
                xsb = xs_tiles[i]
                gt = gt_tiles[i]
                f2p = ps_vv.tile([P, 512], F32, tag="vv")
                for f in range(8):
                    nc.tensor.matmul(f2p[:, 0:256], gt[:, f * P:(f + 1) * P],
                                     w2[:, f, :], start=(f == 0), stop=False)
                nc.tensor.matmul(f2p[:, 0:256], ones1, b2, start=False, stop=False)
                nc.tensor.matmul(f2p[:, 0:256], idn, xsb[:], start=False, stop=True)
                outs = sb.tile([P, D], F32, tag="outs")
                nc.vector.tensor_copy(outs[:], f2p[:, 0:256])
                # dispatch via ACT's DGE: lands right after the copy in ACT's
                # stream, so it never head-of-line blocks SP's input loads
                nc.scalar.dma_start(io["out"][i * P:(i + 1) * P, :], outs[:])
            tc.cur_priority -= 1000000

        def run_ffn_full(blks):
            hts = emit_ffn_ln2(blks, None)
            emit_ffn_ff(blks, hts)

        # Non-uniform superblocks: small terminal group => short final
        # (non-overlappable) FFN drain.
        sizes = []
        rem = n_blocks
        while rem > 0:
            if rem > G:
                sizes.append(G); rem -= G
            elif rem > 3:
                sizes.append(rem - 3); rem = 3
            else:
                sizes.append(rem); rem = 0
        pending = None
        pending_ht = None
        prev_stage = None
        base = 0
        for sz in sizes:
            blocks = list(range(base, base + sz))
            base += sz
            for j, i in enumerate(blocks):
                stg = emit_stage_a(i)
                if prev_stage is not None:
                    emit_stage_b(prev_stage, next_kvs=stg.get("last_kvs"))
                emit_stage_a2(stg)
                prev_stage = stg
                if pending is not None:
                    if j == min(1, len(blocks) - 1):
                        pending_ht = emit_ffn_ln2(pending, None)
                    elif j >= min(FFJ, len(blocks) - 1) and pending_ht is not None:
                        emit_ffn_ff(pending, pending_ht)
                        pending = None
                        pending_ht = None
            if pending is not None:
                if pending_ht is None:
                    pending_ht = emit_ffn_ln2(pending, None)
                emit_ffn_ff(pending, pending_ht)
                pending = None
                pending_ht = None
            pending = blocks
        if prev_stage is not None:
            emit_stage_b(prev_stage)
            prev_stage = None
        if pending is not None:
            run_ffn_full(pending)


def _inst(x):
    return getattr(x, "ins", x)


_ACT_TABLES_PATCHED = False


def _patch_act_tables():
    # Bias bacc's act-table chooser: Ln and Exp both resolve to the
    # natural_log_exp_and_others set (one resident table set for LN-rsqrt
    # and softmax) instead of bouncing between natural_log and exp_and_others.
    global _ACT_TABLES_PATCHED
    if _ACT_TABLES_PATCHED:
        return
    import concourse.hw_specs as _hw
    import concourse.bacc as _bacc_mod
    _orig = _hw.get_activation_tables

    def patched(arch):
        t = dict(_orig(arch))
        exp_t = mybir.ActivationFunctionType.Exp
        ln_t = mybir.ActivationFunctionType.Ln
        for name, fns in t.items():
            if name != "natural_log_exp_and_others" and (
                    exp_t in fns or ln_t in fns):
                t[name] = fns - {exp_t, ln_t}
        return t

    _bacc_mod.get_activation_tables = patched
    _ACT_TABLES_PATCHED = True


def build_bass(n_blocks=NBLK, bc=BC):
    _patch_act_tables()
    nc = bacc.Bacc("TRN2", target_bir_lowering=False, debug=False,
                   num_devices=NCORES)
    io = {}
    io["xab"] = nc.dram_tensor("xab", [bc, D], BF16, kind="ExternalInput").ap()
    io["xt8"] = nc.dram_tensor("xt8", [n_blocks * 2 * P, K * P], FP8,
                               kind="ExternalInput").ap()
    io["wbf"] = nc.dram_tensor("wbf", [P, WBF_COLS], BF16, kind="ExternalInput").ap()
    io["w8"] = nc.dram_tensor("w8", [P, W8_COLS], FP8, kind="ExternalInput").ap()
    io["vrow"] = nc.dram_tensor("vrow", [1, VROW_COLS], BF16, kind="ExternalInput").ap()
    io["out"] = nc.dram_tensor("out", [bc, D], F32, kind="ExternalOutput").ap()
    with tile.TileContext(nc) as tc:
        emit_layer(tc, io, n_blocks)
    nc.compile()
    return nc


_CACHED_NC = None


def get_nc():
    global _CACHED_NC
    if _CACHED_NC is None:
        _CACHED_NC = build_bass()
    return _CACHED_NC


F8NP = ml_dtypes.float8_e4m3fn
BFNP = ml_dtypes.bfloat16


def make_in_maps(inputs, bc=BC, n_blocks=NBLK, ncores=NCORES):
    f = np.float32
    x_anc = np.ascontiguousarray(inputs["x_anc"], dtype=f)
    x_nei = np.ascontiguousarray(inputs["x_nei"], dtype=f)
    ln1_g = np.asarray(inputs["ln1_g"], f)
    ln1_b = np.asarray(inputs["ln1_b"], f)
    ln2_g = np.asarray(inputs["ln2_g"], f)
    ln2_b = np.asarray(inputs["ln2_b"], f)
    W_q = np.asarray(inputs["W_q"], f)
    W_k = np.asarray(inputs["W_k"], f)
    W_v = np.asarray(inputs["W_v"], f)
    W_o = np.asarray(inputs["W_o"], f)
    ff1_w = np.asarray(inputs["ff1_w"], f)
    ff1_b = np.asarray(inputs["ff1_b"], f)
    ff2_w = np.asarray(inputs["ff2_w"], f)
    ff2_b = np.asarray(inputs["ff2_b"], f)
    sc = f(1.0 / math.sqrt(DK))

    wq_f = (ln1_g[:, None] * W_q) * sc          # folded LN1 gain + score scale
    bias_q = (ln1_b @ W_q * sc)[None, :]
    ff1w_f = ln2_g[:, None] * ff1_w             # folded LN2 gain
    bias_ff1 = (ff1_b + ln2_b @ ff1_w)[None, :]

    def chunk2(w):  # [256, n] -> [128, 2, n] (d-chunks in dim1)
        return w.reshape(2, P, -1).transpose(1, 0, 2)

    wbf = np.zeros((P, WBF_COLS), BFNP)
    wbf[:, WQ_OFF:WQ_OFF + 512] = chunk2(wq_f).reshape(P, 512).astype(BFNP)
    wbf[:, WO_OFF:WO_OFF + 512] = chunk2(W_o).reshape(P, 512).astype(BFNP)
    w1p = ff1w_f.reshape(2, P, 8, P).transpose(1, 0, 2, 3)
    wbf[:, W1_OFF:W1_OFF + 2048] = w1p.reshape(P, 2048).astype(BFNP)
    w2p = ff2_w.reshape(8, P, D).transpose(1, 0, 2)
    wbf[:, W2_OFF:W2_OFF + 2048] = w2p.reshape(P, 2048).astype(BFNP)
    wbf[:, IDN_OFF:IDN_OFF + P] = np.eye(P, dtype=BFNP)

    w8 = np.zeros((P, W8_COLS), F8NP)
    w8[:, WK8_OFF:WK8_OFF + 512] = chunk2(W_k).reshape(P, 512).astype(F8NP)
    w8[:, WV8_OFF:WV8_OFF + 512] = chunk2(W_v).reshape(P, 512).astype(F8NP)
    idn2 = np.concatenate([np.eye(P, dtype=F8NP), np.eye(P, dtype=F8NP)], axis=1)
    w8[:, IDN2_OFF:IDN2_OFF + 256] = idn2.reshape(P, 256)

    vrow = np.zeros((1, VROW_COLS), BFNP)
    vrow[0, ONES_OFF:ONES_OFF + P] = 1.0
    vrow[0, BQ_OFF:BQ_OFF + 256] = bias_q[0].astype(BFNP)
    vrow[0, B2_OFF:B2_OFF + 256] = ff2_b.astype(BFNP)
    vrow[0, B1_OFF:B1_OFF + 1024] = bias_ff1[0].astype(BFNP)

    shared = {"wbf": wbf, "w8": w8, "vrow": vrow}

    xn8 = x_nei.astype(F8NP)                    # [B, K, D]
    in_maps = []
    for c in range(ncores):
        sl = slice(c * bc, (c + 1) * bc)
        m = dict(shared)
        m["xab"] = np.ascontiguousarray(x_anc[sl].astype(BFNP))
        xb = xn8[sl].reshape(n_blocks, P, K, 2, P)     # [blk, b, k, c, p]
        xt = xb.transpose(0, 3, 4, 2, 1)               # [blk, c, p, k, b]
        m["xt8"] = np.ascontiguousarray(xt.reshape(n_blocks * 2 * P, K * P))
        in_maps.append(m)
    return in_maps


def kernel(**inputs):
    from concourse.bass_utils import run_bass_kernel_spmd

    in_maps = make_in_maps(inputs)
    res = run_bass_kernel_spmd(get_nc(), in_maps, core_ids=list(range(NCORES)))
    return np.concatenate([res.results[c]["out"] for c in range(NCORES)], axis=0)

